# revision 1
# baseline (speedup 1.0000x reference)
"""KWS-SNN Trainium2 kernel: 8-way batch-parallel Bass/Tile implementation.

Per core (BC=64): mean over T -> conv1(block-diag batch-packed, K=72,M=128)
-> BN+ReLU+pool (free-dim) -> conv2 (9 tap-matmuls, K=64, M=128, SBUF-shift
rhs) -> BN+ReLU+pool -> fc1 (K-tiled GEMM, fp32r) -> transpose -> 25-step
LIF scan -> spikes out.
"""
import sys
sys.path.insert(0, '/opt/trn_rl_repo')
import numpy as np
import concourse.bass as bass
import concourse.mybir as mybir
import bass_rust
from concourse.tile import TileContext
from concourse import bass_utils

F32 = mybir.dt.float32
F32R = mybir.dt.float32r
AL = mybir.AluOpType
AF = mybir.ActivationFunctionType

T, BF, H, W = 25, 512, 100, 64
NCORE = 8
BC = BF // NCORE          # 64 batches per core
NCL = 35

# padded geometry
XMP_B = 102 * 66          # 6732 per-batch padded mean image
F1P_C = 52 * 34           # 1768 per-channel padded conv1 output
F1P_B = 16 * F1P_C        # 28288 per-batch


def rap(handle, off, dims):
    a = handle.ap()
    return bass_rust.AP(tensor=a.tensor, offset=off, ap=[list(d) for d in dims])


def split_multi_waits(nc, max_waits=1):
    """This walrus build rejects >1 sync-wait per instruction; hoist excess
    waits onto same-engine NoOps inserted immediately before."""
    ctr = 0
    for f in nc.m.functions:
        for bb in f.blocks:
            il = bb.instructions
            if not any(i.sync_info is not None and len(i.sync_info.on_wait) > max_waits
                       for i in il):
                continue
            new = []
            for inst in il:
                si = inst.sync_info
                if si is not None and len(si.on_wait) > max_waits:
                    waits = list(si.on_wait)
                    while len(waits) > max_waits:
                        w = waits.pop(0)
                        nop = mybir.InstNoOp(
                            name=f"_ws_{ctr}", engine=inst.engine,
                            sync_info=mybir.SyncInfo(on_wait=[w], on_update=[]),
                            bass_nofuse=True)
                        ctr += 1
                        new.append(nop)
                    inst.sync_info = mybir.SyncInfo(
                        on_wait=waits, on_update=list(si.on_update))
                new.append(inst)
            bb.instructions = new
    return ctr


def build(be1, be2, be3):
    nc = bass.Bass()
    xin = nc.dram_tensor("x", [T, BC, H, W], F32, kind="ExternalInput")
    w1b = nc.dram_tensor("w1b", [72, 128], F32, kind="ExternalInput")
    bn1s = nc.dram_tensor("bn1s", [128], F32, kind="ExternalInput")
    bn1b = nc.dram_tensor("bn1b", [128], F32, kind="ExternalInput")
    w2t = nc.dram_tensor("w2t", [9, 64, 128], F32, kind="ExternalInput")
    bn2s = nc.dram_tensor("bn2s", [128], F32, kind="ExternalInput")
    bn2b = nc.dram_tensor("bn2b", [128], F32, kind="ExternalInput")
    w1t = nc.dram_tensor("w1t", [12800, 256], F32, kind="ExternalInput")
    b1 = nc.dram_tensor("b1", [256], F32, kind="ExternalInput")
    w2a = nc.dram_tensor("w2a", [128, 128], F32, kind="ExternalInput")
    w2b = nc.dram_tensor("w2b", [128, 128], F32, kind="ExternalInput")
    b2 = nc.dram_tensor("b2", [128], F32, kind="ExternalInput")
    w3t = nc.dram_tensor("w3t", [128, 35], F32, kind="ExternalInput")
    b3 = nc.dram_tensor("b3", [35], F32, kind="ExternalInput")
    ident = nc.dram_tensor("ident", [64, 64], F32, kind="ExternalInput")

    xm = nc.dram_tensor("xm", [BC * 6400], F32, kind="Internal")
    xmp = nc.dram_tensor("xmp", [BC * XMP_B], F32, kind="Internal")
    f1p = nc.dram_tensor("f1p", [BC * F1P_B], F32, kind="Internal")
    featd = nc.dram_tensor("featd", [BC * 12800], F32, kind="Internal")
    out = nc.dram_tensor("out", [T, BC, NCL], F32, kind="ExternalOutput")
    DBG = bool(__import__("os").environ.get("SNN_DBG"))
    if DBG:
        dxm = nc.dram_tensor("dxm", [BC * 6400], F32, kind="ExternalOutput")
        df1 = nc.dram_tensor("df1", [BC * F1P_B], F32, kind="ExternalOutput")
        dft = nc.dram_tensor("dft", [BC * 12800], F32, kind="ExternalOutput")
        dc1 = nc.dram_tensor("dc1", [128, 128], F32, kind="ExternalOutput")

    with TileContext(nc) as tc:
        with (
            tc.tile_pool(name="const", bufs=1) as pc,
            tc.tile_pool(name="main", bufs=1) as pm,
            tc.tile_pool(name="psA", bufs=4, space="PSUM") as psA,
            tc.tile_pool(name="psB", bufs=4, space="PSUM") as psB,
        ):
            # ---- constants to SBUF ----
            w1b_sb = pc.tile([72, 128], F32, tag="w1b")
            nc.sync.dma_start(w1b_sb[:], w1b.ap())
            w2t_sb = pc.tile([64, 9 * 128], F32, tag="w2t")
            nc.sync.dma_start(w2t_sb[:], rap(w2t, 0, [[128, 64], [8192, 9], [1, 128]]))
            bn1s_sb = pc.tile([128, 1], F32, tag="b1s")
            bn1b_sb = pc.tile([128, 1], F32, tag="b1b")
            bn2s_sb = pc.tile([128, 1], F32, tag="b2s")
            bn2b_sb = pc.tile([128, 1], F32, tag="b2b")
            for sb, dr in ((bn1s_sb, bn1s), (bn1b_sb, bn1b),
                           (bn2s_sb, bn2s), (bn2b_sb, bn2b)):
                nc.sync.dma_start(sb[:], rap(dr, 0, [[1, 128], [1, 1]]))
            w2a_sb = pc.tile([128, 128], F32, tag="w2a")
            nc.sync.dma_start(w2a_sb[:], w2a.ap())
            w2b_sb = pc.tile([128, 128], F32, tag="w2b")
            nc.sync.dma_start(w2b_sb[:], w2b.ap())
            w3t_sb = pc.tile([128, 35], F32, tag="w3t")
            nc.sync.dma_start(w3t_sb[:], w3t.ap())
            b1_sb = pc.tile([128, 2], F32, tag="fb1")
            nc.sync.dma_start(b1_sb[:], rap(b1, 0, [[1, 128], [128, 2]]))
            b2_sb = pc.tile([128, 1], F32, tag="fb2")
            nc.sync.dma_start(b2_sb[:], rap(b2, 0, [[1, 128], [1, 1]]))
            b3_sb = pc.tile([35, 1], F32, tag="fb3")
            nc.sync.dma_start(b3_sb[:], rap(b3, 0, [[1, 35], [1, 1]]))
            id_sb = pc.tile([64, 64], F32, tag="id")
            nc.sync.dma_start(id_sb[:], ident.ap())

            cur1T = pm.tile([128, 128], F32, tag="cur1T")
            outsb = pm.tile([35, T * 64], F32, tag="outsb")

            # ---- phase A: mean over T (sum; /25 folded into conv1 w) ----
            with tc.tile_pool(name="phA", bufs=2) as pa:
                acc = pm.tile([128, 3200], F32, tag="acc")
                for t in range(T):
                    xt = pa.tile([128, 3200], F32, tag="xt")
                    nc.sync.dma_start(
                        xt[:], rap(xin, t * 409600, [[3200, 128], [1, 3200]]))
                    if t == 0:
                        nc.vector.tensor_copy(acc[:], xt[:])
                    else:
                        nc.vector.tensor_add(acc[:], acc[:], xt[:])

                # zero-fill pads
                zt = pa.tile([128, 3536], F32, tag="zt")
                nc.gpsimd.memset(zt[:], 0.0)
                nc.sync.dma_start(
                    rap(xmp, 0, [[3366, 128], [1, 3366]]), zt[:, 0:3366])
                for i in range(4):
                    nc.sync.dma_start(
                        rap(f1p, i * 452608, [[3536, 128], [1, 3536]]),
                        zt[:, 0:3536])
                # write mean (sum) flat, then pad via DRAM->DRAM copy
                nc.sync.dma_start(
                    rap(xm, 0, [[3200, 128], [1, 3200]]), acc[:])
                nc.sync.dma_start(
                    rap(xmp, 67, [[XMP_B, 64], [66, 100], [1, 64]]),
                    rap(xm, 0, [[1, BC * 6400]]))

            # ---- phase C: conv1 + bn + relu + pool, 8 chunks of 8 batches ----
            with tc.tile_pool(name="phC", bufs=2) as p1:
                for c in range(8):
                    im1 = p1.tile([72, 6400], F32, tag="im1")
                    for dh in range(3):
                        for dw in range(3):
                            nc.sync.dma_start(
                                im1[dh * 24 + dw * 8:dh * 24 + dw * 8 + 8, :],
                                rap(xmp, c * 8 * XMP_B + dh * 66 + dw,
                                    [[XMP_B, 8], [66, 100], [1, 64]]))
                    wm = p1.tile([128, 3200], F32, tag="wm")
                    for s in range(13):
                        n = 512 if s < 12 else 256
                        ps = psA.tile([128, 512], F32, tag="cv")
                        nc.tensor.matmul(
                            ps[:, 0:n], w1b_sb[:],
                            im1[:, s * 512:s * 512 + n],
                            start=True, stop=True)
                        cp = p1.tile([128, 512], F32, tag="cp")
                        nc.scalar.copy(cp[:, 0:n], ps[:, 0:n])
                        pv = cp[:, 0:n].rearrange("p (h w t) -> p h w t",
                                                  w=32, t=2)
                        wv = wm[:, s * 256:s * 256 + n // 2].rearrange(
                            "p (h w) -> p h w", w=32)
                        nc.vector.tensor_max(
                            wv[:, :, :], pv[:, :, :, 0], pv[:, :, :, 1])
                    ac = p1.tile([128, 3200], F32, tag="ac")
                    nc.scalar.activation(ac[:], wm[:], AF.Relu,
                                         bias=bn1b_sb[:, 0:1],
                                         scale=bn1s_sb[:, 0:1])
                    hp = p1.tile([128, 1600], F32, tag="hp")
                    av = ac[:].rearrange("p (r t w) -> p r t w", t=2, w=32)
                    nc.vector.tensor_max(
                        hp[:].rearrange("p (r w) -> p r w", w=32),
                        av[:, :, 0, :], av[:, :, 1, :])
                    for bg in range(8):
                        nc.sync.dma_start(
                            rap(f1p, (c * 8 + bg) * F1P_B + 35,
                                [[F1P_C, 16], [34, 50], [1, 32]]),
                            hp[bg * 16:(bg + 1) * 16, :])

            # ---- phase D: conv2 + bn + relu + pool, 16 groups of 4 ----
            with tc.tile_pool(name="phD", bufs=2) as p2:
                rblk = [(0, 13), (13, 13), (26, 12), (38, 12)]
                for g in range(16):
                    fq = p2.tile([64, F1P_C], F32, tag="fq")
                    nc.sync.dma_start(
                        fq[:],
                        rap(f1p, g * 4 * F1P_B,
                            [[F1P_B, 4], [F1P_C, 16], [1, F1P_C]]))
                    fqv = fq[:].rearrange("p (r w) -> p r w", w=34)
                    wm2 = p2.tile([128, 800], F32, tag="wm2")
                    for bi, (r0, nr) in enumerate(rblk):
                        ps = psA.tile([128, 512], F32, tag="cv")
                        n = nr * 32
                        for ti in range(9):
                            dh, dw = ti // 3, ti % 3
                            nc.tensor.matmul(
                                ps[:, 0:n],
                                w2t_sb[:, ti * 128:(ti + 1) * 128],
                                fqv[:, dh + r0:dh + r0 + nr,
                                    dw:dw + 32],
                                start=(ti == 0), stop=(ti == 8))
                        cp2 = p2.tile([128, 512], F32, tag="cp2")
                        nc.scalar.copy(cp2[:, 0:n], ps[:, 0:n])
                        pv = cp2[:, 0:n].rearrange("p (r w t) -> p r w t",
                                                  w=16, t=2)
                        wv2 = wm2[:].rearrange("p (r w) -> p r w", w=16)
                        nc.vector.tensor_max(
                            wv2[:, r0:r0 + nr, :],
                            pv[:, :, :, 0], pv[:, :, :, 1])
                    ac2 = p2.tile([128, 800], F32, tag="ac2")
                    nc.scalar.activation(ac2[:], wm2[:], AF.Relu,
                                         bias=bn2b_sb[:, 0:1],
                                         scale=bn2s_sb[:, 0:1])
                    hp2 = p2.tile([128, 400], F32, tag="hp2")
                    a2v = ac2[:].rearrange("p (r t w) -> p r t w", t=2, w=16)
                    nc.vector.tensor_max(
                        hp2[:].rearrange("p (r w) -> p r w", w=16),
                        a2v[:, :, 0, :], a2v[:, :, 1, :])
                    for bg in range(4):
                        nc.sync.dma_start(
                            rap(featd, (g * 4 + bg) * 12800,
                                [[400, 32], [16, 25], [1, 16]]),
                            hp2[bg * 32:(bg + 1) * 32, :])

            # ---- phase E: fc1 GEMM (K=12800 in 100 tiles) + transpose ----
            with tc.tile_pool(name="phE", bufs=4) as p4:
                psf = psB.tile([64, 256], F32, tag="b")
                for k in range(100):
                    ft = p4.tile([128, 64], F32, tag="ft")
                    nc.sync.dma_start(
                        ft[:], rap(featd, k * 128, [[1, 128], [12800, 64]]))
                    wt = p4.tile([128, 256], F32, tag="wt")
                    nc.sync.dma_start(
                        wt[:], rap(w1t, k * 128 * 256, [[256, 128], [1, 256]]))
                    nc.tensor.matmul(psf[:], ft[:],
                                     wt[:],
                                     start=(k == 0), stop=(k == 99))
                cur1 = p4.tile([64, 256], F32, tag="cur1")
                nc.scalar.copy(cur1[:], psf[:])
                for h in range(2):
                    pst = psB.tile([128, 64], F32, tag="b")
                    nc.tensor.transpose(pst[:], cur1[:, h * 128:(h + 1) * 128],
                                        id_sb[:])
                    nc.vector.tensor_scalar(cur1T[:, h * 64:(h + 1) * 64],
                                            pst[:], b1_sb[:, h:h + 1], None,
                                            AL.add)

            # ---- phase F: LIF scan ----
            with tc.tile_pool(name="phF", bufs=3) as p5:
                m1 = pm.tile([128, 128], F32, tag="m1")
                m2 = pm.tile([128, 64], F32, tag="m2")
                m3 = pm.tile([35, 64], F32, tag="m3")
                nc.gpsimd.memset(m1[:], 0.0)
                nc.gpsimd.memset(m2[:], 0.0)
                nc.gpsimd.memset(m3[:], 0.0)
                for t in range(T):
                    r1 = p5.tile([128, 128], F32, tag="r1")
                    nc.vector.tensor_scalar(r1[:], m1[:], 1.0, None, AL.is_gt)
                    nc.vector.scalar_tensor_tensor(
                        m1[:], m1[:], be1, cur1T[:], AL.mult, AL.add)
                    nc.vector.tensor_sub(m1[:], m1[:], r1[:])
                    s1 = p5.tile([128, 128], F32, tag="s1")
                    nc.vector.tensor_scalar(s1[:], m1[:], 1.0, None, AL.is_gt)
                    ps2 = psB.tile([128, 64], F32, tag="b")
                    nc.tensor.matmul(ps2[:], w2a_sb[:],
                                     s1[:, 0:64],
                                     start=True, stop=False)
                    nc.tensor.matmul(ps2[:], w2b_sb[:],
                                     s1[:, 64:128],
                                     start=False, stop=True)
                    r2 = p5.tile([128, 64], F32, tag="r2")
                    nc.vector.tensor_scalar(r2[:], m2[:], 1.0, None, AL.is_gt)
                    nc.vector.scalar_tensor_tensor(
                        m2[:], m2[:], be2, ps2[:], AL.mult, AL.add)
                    nc.vector.tensor_sub(m2[:], m2[:], r2[:])
                    nc.vector.tensor_scalar(m2[:], m2[:], b2_sb[:, 0:1],
                                            None, AL.add)
                    s2 = p5.tile([128, 64], F32, tag="s2")
                    nc.vector.tensor_scalar(s2[:], m2[:], 1.0, None, AL.is_gt)
                    ps3 = psB.tile([35, 64], F32, tag="b")
                    nc.tensor.matmul(ps3[:], w3t_sb[:],
                                     s2[:],
                                     start=True, stop=True)
                    r3 = p5.tile([35, 64], F32, tag="r3")
                    nc.vector.tensor_scalar(r3[:], m3[:], 1.0, None, AL.is_gt)
                    nc.vector.scalar_tensor_tensor(
                        m3[:], m3[:], be3, ps3[:], AL.mult, AL.add)
                    nc.vector.tensor_sub(m3[:], m3[:], r3[:])
                    nc.vector.tensor_scalar(m3[:], m3[:], b3_sb[:, 0:1],
                                            None, AL.add)
                    nc.vector.tensor_scalar(outsb[:, t * 64:(t + 1) * 64],
                                            m3[:], 1.0, None, AL.is_gt)
                nc.sync.dma_start(
                    rap(out, 0, [[1, 35], [BC * 35, 25], [35, 64]]), outsb[:])
            if DBG:
                nc.sync.dma_start(dxm.ap(), xm.ap())
                nc.sync.dma_start(df1.ap(), f1p.ap())
                nc.sync.dma_start(dft.ap(), featd.ap())
                nc.sync.dma_start(dc1.ap(), cur1T[:])

    split_multi_waits(nc)
    return nc


def prep(inputs):
    f = np.float32
    w1 = np.asarray(inputs["conv1_w"], f)
    s1v = np.asarray(inputs["bn1_g"], f) / np.sqrt(
        np.asarray(inputs["bn1_v"], f) + 1e-5)
    sh1 = np.asarray(inputs["bn1_b"], f) + (
        np.asarray(inputs["conv1_b"], f) - np.asarray(inputs["bn1_m"], f)) * s1v
    w2 = np.asarray(inputs["conv2_w"], f)
    s2v = np.asarray(inputs["bn2_g"], f) / np.sqrt(
        np.asarray(inputs["bn2_v"], f) + 1e-5)
    sh2 = np.asarray(inputs["bn2_b"], f) + (
        np.asarray(inputs["conv2_b"], f) - np.asarray(inputs["bn2_m"], f)) * s2v

    w1b = np.zeros((72, 128), f)
    for bg in range(8):
        for ch in range(16):
            for dh in range(3):
                for dw in range(3):
                    w1b[dh * 24 + dw * 8 + bg, bg * 16 + ch] = \
                        w1[ch, 0, dh, dw] / 25.0
    bn1sv = np.tile(s1v, 8).astype(f)
    bn1bv = np.tile(sh1, 8).astype(f)

    w2t9 = np.zeros((9, 64, 128), f)
    for ti in range(9):
        dh, dw = ti // 3, ti % 3
        for bg in range(4):
            for ci in range(16):
                for co in range(32):
                    w2t9[ti, bg * 16 + ci, bg * 32 + co] = w2[co, ci, dh, dw]
    bn2sv = np.tile(s2v, 4).astype(f)
    bn2bv = np.tile(sh2, 4).astype(f)

    return dict(
        w1b=w1b, bn1s=bn1sv, bn1b=bn1bv,
        w2t=w2t9, bn2s=bn2sv, bn2b=bn2bv,
        w1t=np.ascontiguousarray(np.asarray(inputs["fc1_w"], f).T),
        b1=np.asarray(inputs["fc1_b"], f),
        w2a=np.ascontiguousarray(np.asarray(inputs["fc2_w"], f).T[0:128]),
        w2b=np.ascontiguousarray(np.asarray(inputs["fc2_w"], f).T[128:256]),
        b2=np.asarray(inputs["fc2_b"], f),
        w3t=np.ascontiguousarray(np.asarray(inputs["fc3_w"], f).T),
        b3=np.asarray(inputs["fc3_b"], f),
        ident=np.eye(64, dtype=f),
    )


def kernel(**inputs):
    f = np.float32
    x = np.asarray(inputs["x"], f)
    be1 = float(np.clip(np.asarray(inputs["beta1"], f), 0.0, 1.0))
    be2 = float(np.clip(np.asarray(inputs["beta2"], f), 0.0, 1.0))
    be3 = float(np.clip(np.asarray(inputs["beta3"], f), 0.0, 1.0))
    consts = prep(inputs)
    nc = build(be1, be2, be3)
    in_maps = []
    for c in range(NCORE):
        m = {"x": np.ascontiguousarray(x[:, c * BC:(c + 1) * BC])}
        m.update(consts)
        in_maps.append(m)
    res = bass_utils.run_bass_kernel_spmd(nc, in_maps, core_ids=list(range(NCORE)))
    return np.concatenate([res.results[c]["out"] for c in range(NCORE)], axis=1)



# revision 10
# speedup vs baseline: 3592.8371x; 3592.8371x over previous
"""KWS-SNN Trainium2 kernel: 8-way batch-parallel Bass/Tile implementation.

Per core (BC=64): mean over T -> conv1(block-diag batch-packed, K=72,M=128)
-> BN+ReLU+pool (free-dim) -> conv2 (9 tap-matmuls, K=64, M=128, SBUF-shift
rhs) -> BN+ReLU+pool -> fc1 (K-tiled GEMM, fp32r) -> transpose -> 25-step
LIF scan -> spikes out.
"""
import sys
sys.path.insert(0, '/opt/trn_rl_repo')
import numpy as np
import concourse.bass as bass
import concourse.mybir as mybir
import bass_rust
from concourse.tile import TileContext
from concourse import bass_utils

F32 = mybir.dt.float32
F32R = mybir.dt.float32r
AL = mybir.AluOpType
AF = mybir.ActivationFunctionType

T, BF, H, W = 25, 512, 100, 64
NCORE = 8
BC = BF // NCORE          # 64 batches per core
NCL = 35

# padded geometry
XMP_B = 102 * 66          # 6732 per-batch padded mean image
F1P_C = 52 * 34           # 1768 per-channel padded conv1 output
F1P_B = 16 * F1P_C        # 28288 per-batch


def rap(handle, off, dims, dt=None):
    a = handle.ap()
    if dt is not None:
        a = a.bitcast(dt)
    return bass_rust.AP(tensor=a.tensor, offset=off, ap=[list(d) for d in dims])


def split_multi_waits(nc, max_waits=1):
    """This walrus build rejects >1 sync-wait per instruction; hoist excess
    waits onto same-engine NoOps inserted immediately before."""
    ctr = 0
    for f in nc.m.functions:
        for bb in f.blocks:
            il = bb.instructions
            if not any(i.sync_info is not None and len(i.sync_info.on_wait) > max_waits
                       for i in il):
                continue
            new = []
            for inst in il:
                si = inst.sync_info
                if si is not None and len(si.on_wait) > max_waits:
                    waits = list(si.on_wait)
                    while len(waits) > max_waits:
                        w = waits.pop(0)
                        nop = mybir.InstNoOp(
                            name=f"_ws_{ctr}", engine=inst.engine,
                            sync_info=mybir.SyncInfo(on_wait=[w], on_update=[]),
                            bass_nofuse=True)
                        ctr += 1
                        new.append(nop)
                    inst.sync_info = mybir.SyncInfo(
                        on_wait=waits, on_update=list(si.on_update))
                new.append(inst)
            bb.instructions = new
    return ctr


def build(be1, be2, be3):
    nc = bass.Bass()
    xin = nc.dram_tensor("x", [T, BC, H, W], F32, kind="ExternalInput")
    w1b = nc.dram_tensor("w1b", [72, 128], F32, kind="ExternalInput")
    bn1s = nc.dram_tensor("bn1s", [128], F32, kind="ExternalInput")
    bn1b = nc.dram_tensor("bn1b", [128], F32, kind="ExternalInput")
    w2t = nc.dram_tensor("w2t", [9, 64, 128], F32, kind="ExternalInput")
    bn2s = nc.dram_tensor("bn2s", [128], F32, kind="ExternalInput")
    bn2b = nc.dram_tensor("bn2b", [128], F32, kind="ExternalInput")
    w1t = nc.dram_tensor("w1t", [12800, 256], F32, kind="ExternalInput")
    b1 = nc.dram_tensor("b1", [256], F32, kind="ExternalInput")
    w2a = nc.dram_tensor("w2a", [128, 128], F32, kind="ExternalInput")
    w2b = nc.dram_tensor("w2b", [128, 128], F32, kind="ExternalInput")
    b2 = nc.dram_tensor("b2", [128], F32, kind="ExternalInput")
    w3t = nc.dram_tensor("w3t", [128, 35], F32, kind="ExternalInput")
    b3 = nc.dram_tensor("b3", [35], F32, kind="ExternalInput")
    ident = nc.dram_tensor("ident", [64, 64], F32, kind="ExternalInput")

    xm = nc.dram_tensor("xm", [BC * 6400], F32, kind="Internal")
    xmp = nc.dram_tensor("xmp", [BC * XMP_B], F32, kind="Internal")
    f1p = nc.dram_tensor("f1p", [BC * F1P_B], F32, kind="Internal")
    featd = nc.dram_tensor("featd", [BC * 12800], F32, kind="Internal")
    out = nc.dram_tensor("out", [T, BC, NCL], F32, kind="ExternalOutput")
    DBG = bool(__import__("os").environ.get("SNN_DBG"))
    if DBG:
        dxm = nc.dram_tensor("dxm", [BC * 6400], F32, kind="ExternalOutput")
        df1 = nc.dram_tensor("df1", [BC * F1P_B], F32, kind="ExternalOutput")
        dft = nc.dram_tensor("dft", [BC * 12800], F32, kind="ExternalOutput")
        dc1 = nc.dram_tensor("dc1", [128, 128], F32, kind="ExternalOutput")

    with TileContext(nc) as tc:
        with (
            tc.tile_pool(name="const", bufs=1) as pc,
            tc.tile_pool(name="main", bufs=1) as pm,
            tc.tile_pool(name="psA", bufs=4, space="PSUM") as psA,
            tc.tile_pool(name="psB", bufs=4, space="PSUM") as psB,
        ):
            # ---- constants to SBUF ----
            w1b_sb = pc.tile([72, 128], F32R, tag="w1b")
            nc.sync.dma_start(w1b_sb[:], w1b.ap().bitcast(F32R))
            w2t_sb = pc.tile([64, 9 * 128], F32R, tag="w2t")
            nc.sync.dma_start(
                w2t_sb[:], rap(w2t, 0, [[128, 64], [8192, 9], [1, 128]], F32R))
            bn1s_sb = pc.tile([128, 1], F32, tag="b1s")
            bn1b_sb = pc.tile([128, 1], F32, tag="b1b")
            bn2s_sb = pc.tile([128, 1], F32, tag="b2s")
            bn2b_sb = pc.tile([128, 1], F32, tag="b2b")
            for sb, dr in ((bn1s_sb, bn1s), (bn1b_sb, bn1b),
                           (bn2s_sb, bn2s), (bn2b_sb, bn2b)):
                nc.sync.dma_start(sb[:], rap(dr, 0, [[1, 128], [1, 1]]))
            w2a_sb = pc.tile([128, 128], F32R, tag="w2a")
            nc.sync.dma_start(w2a_sb[:], w2a.ap().bitcast(F32R))
            w2b_sb = pc.tile([128, 128], F32R, tag="w2b")
            nc.sync.dma_start(w2b_sb[:], w2b.ap().bitcast(F32R))
            w3t_sb = pc.tile([128, 35], F32R, tag="w3t")
            nc.sync.dma_start(w3t_sb[:], w3t.ap().bitcast(F32R))
            b1_sb = pc.tile([128, 2], F32, tag="fb1")
            nc.sync.dma_start(b1_sb[:], rap(b1, 0, [[1, 128], [128, 2]]))
            b2_sb = pc.tile([128, 1], F32, tag="fb2")
            nc.sync.dma_start(b2_sb[:], rap(b2, 0, [[1, 128], [1, 1]]))
            b3_sb = pc.tile([35, 1], F32, tag="fb3")
            nc.sync.dma_start(b3_sb[:], rap(b3, 0, [[1, 35], [1, 1]]))
            id_sb = pc.tile([64, 64], F32, tag="id")
            nc.sync.dma_start(id_sb[:], ident.ap())

            cur1T = pm.tile([128, 128], F32, tag="cur1T")
            outsb = pm.tile([35, T * 64], F32, tag="outsb")

            # ---- phase A: mean over T (sum; /25 folded into conv1 w) ----
            with tc.tile_pool(name="phA", bufs=2) as pa:
                acc = pm.tile([128, 3200], F32, tag="acc")
                for t in range(T):
                    xt = pa.tile([128, 3200], F32, tag="xt")
                    nc.sync.dma_start(
                        xt[:], rap(xin, t * 409600, [[3200, 128], [1, 3200]]))
                    if t == 0:
                        nc.vector.tensor_copy(acc[:], xt[:])
                    else:
                        nc.vector.tensor_add(acc[:], acc[:], xt[:])

                # zero-fill pads
                zt = pa.tile([128, 3536], F32, tag="zt")
                nc.gpsimd.memset(zt[:], 0.0)
                nc.sync.dma_start(
                    rap(xmp, 0, [[3366, 128], [1, 3366]]), zt[:, 0:3366])
                for i in range(4):
                    nc.sync.dma_start(
                        rap(f1p, i * 452608, [[3536, 128], [1, 3536]]),
                        zt[:, 0:3536])
                # write mean (sum) flat, then pad via DRAM->DRAM copy
                nc.sync.dma_start(
                    rap(xm, 0, [[3200, 128], [1, 3200]]), acc[:])
                nc.sync.dma_start(
                    rap(xmp, 67, [[XMP_B, 64], [66, 100], [1, 64]]),
                    rap(xm, 0, [[1, BC * 6400]]))

            # ---- phase C: conv1 + bn + relu + pool, 8 chunks of 8 batches ----
            with tc.tile_pool(name="phC", bufs=2) as p1:
                for c in range(8):
                    im1 = p1.tile([72, 6400], F32R, tag="im1")
                    for dh in range(3):
                        for dw in range(3):
                            nc.sync.dma_start(
                                im1[dh * 24 + dw * 8:dh * 24 + dw * 8 + 8, :],
                                rap(xmp, c * 8 * XMP_B + dh * 66 + dw,
                                    [[XMP_B, 8], [66, 100], [1, 64]], F32R))
                    wm = p1.tile([128, 3200], F32, tag="wm")
                    for s in range(13):
                        n = 512 if s < 12 else 256
                        ps = psA.tile([128, 512], F32, tag="cv")
                        nc.tensor.matmul(
                            ps[:, 0:n], w1b_sb[:],
                            im1[:, s * 512:s * 512 + n],
                            start=True, stop=True)
                        cp = p1.tile([128, 512], F32, tag="cp")
                        nc.scalar.copy(cp[:, 0:n], ps[:, 0:n])
                        pv = cp[:, 0:n].rearrange("p (h w t) -> p h w t",
                                                  w=32, t=2)
                        wv = wm[:, s * 256:s * 256 + n // 2].rearrange(
                            "p (h w) -> p h w", w=32)
                        nc.vector.tensor_max(
                            wv[:, :, :], pv[:, :, :, 0], pv[:, :, :, 1])
                    ac = p1.tile([128, 3200], F32, tag="ac")
                    nc.scalar.activation(ac[:], wm[:], AF.Relu,
                                         bias=bn1b_sb[:, 0:1],
                                         scale=bn1s_sb[:, 0:1])
                    hp = p1.tile([128, 1600], F32, tag="hp")
                    av = ac[:].rearrange("p (r t w) -> p r t w", t=2, w=32)
                    nc.vector.tensor_max(
                        hp[:].rearrange("p (r w) -> p r w", w=32),
                        av[:, :, 0, :], av[:, :, 1, :])
                    for bg in range(8):
                        nc.sync.dma_start(
                            rap(f1p, (c * 8 + bg) * F1P_B + 35,
                                [[F1P_C, 16], [34, 50], [1, 32]]),
                            hp[bg * 16:(bg + 1) * 16, :])

            # ---- phase D: conv2 + bn + relu + pool, 16 groups of 4 ----
            with tc.tile_pool(name="phD", bufs=2) as p2:
                rblk = [(0, 13), (13, 13), (26, 12), (38, 12)]
                for g in range(16):
                    fq = p2.tile([64, F1P_C], F32R, tag="fq")
                    nc.sync.dma_start(
                        fq[:],
                        rap(f1p, g * 4 * F1P_B,
                            [[F1P_B, 4], [F1P_C, 16], [1, F1P_C]], F32R))
                    fqv = fq[:].rearrange("p (r w) -> p r w", w=34)
                    wm2 = p2.tile([128, 800], F32, tag="wm2")
                    for bi, (r0, nr) in enumerate(rblk):
                        ps = psA.tile([128, 512], F32, tag="cv")
                        n = nr * 32
                        for ti in range(9):
                            dh, dw = ti // 3, ti % 3
                            nc.tensor.matmul(
                                ps[:, 0:n],
                                w2t_sb[:, ti * 128:(ti + 1) * 128],
                                fqv[:, dh + r0:dh + r0 + nr,
                                    dw:dw + 32],
                                start=(ti == 0), stop=(ti == 8))
                        cp2 = p2.tile([128, 512], F32, tag="cp2")
                        nc.scalar.copy(cp2[:, 0:n], ps[:, 0:n])
                        pv = cp2[:, 0:n].rearrange("p (r w t) -> p r w t",
                                                  w=16, t=2)
                        wv2 = wm2[:].rearrange("p (r w) -> p r w", w=16)
                        nc.vector.tensor_max(
                            wv2[:, r0:r0 + nr, :],
                            pv[:, :, :, 0], pv[:, :, :, 1])
                    ac2 = p2.tile([128, 800], F32, tag="ac2")
                    nc.scalar.activation(ac2[:], wm2[:], AF.Relu,
                                         bias=bn2b_sb[:, 0:1],
                                         scale=bn2s_sb[:, 0:1])
                    hp2 = p2.tile([128, 400], F32, tag="hp2")
                    a2v = ac2[:].rearrange("p (r t w) -> p r t w", t=2, w=16)
                    nc.vector.tensor_max(
                        hp2[:].rearrange("p (r w) -> p r w", w=16),
                        a2v[:, :, 0, :], a2v[:, :, 1, :])
                    for bg in range(4):
                        nc.sync.dma_start(
                            rap(featd, (g * 4 + bg) * 12800,
                                [[400, 32], [16, 25], [1, 16]]),
                            hp2[bg * 32:(bg + 1) * 32, :])

            # ---- phase E: fc1 GEMM (K=12800 in 100 tiles) + transpose ----
            with tc.tile_pool(name="phE", bufs=4) as p4:
                psf = psB.tile([64, 256], F32, tag="b")
                for k in range(100):
                    ft = p4.tile([128, 64], F32R, tag="ft")
                    nc.sync.dma_start(
                        ft[:], rap(featd, k * 128, [[1, 128], [12800, 64]], F32R))
                    wt = p4.tile([128, 256], F32R, tag="wt")
                    nc.sync.dma_start(
                        wt[:], rap(w1t, k * 128 * 256, [[256, 128], [1, 256]], F32R))
                    nc.tensor.matmul(psf[:], ft[:],
                                     wt[:],
                                     start=(k == 0), stop=(k == 99))
                cur1 = p4.tile([64, 256], F32, tag="cur1")
                nc.scalar.copy(cur1[:], psf[:])
                for h in range(2):
                    pst = psB.tile([128, 64], F32, tag="b")
                    nc.tensor.transpose(pst[:], cur1[:, h * 128:(h + 1) * 128],
                                        id_sb[:])
                    nc.vector.tensor_scalar(cur1T[:, h * 64:(h + 1) * 64],
                                            pst[:], b1_sb[:, h:h + 1], None,
                                            AL.add)

            # ---- phase F: LIF scan ----
            with tc.tile_pool(name="phF", bufs=3) as p5:
                m1 = pm.tile([128, 128], F32, tag="m1")
                m2 = pm.tile([128, 64], F32, tag="m2")
                m3 = pm.tile([35, 64], F32, tag="m3")
                nc.gpsimd.memset(m1[:], 0.0)
                nc.gpsimd.memset(m2[:], 0.0)
                nc.gpsimd.memset(m3[:], 0.0)
                for t in range(T):
                    r1 = p5.tile([128, 128], F32, tag="r1")
                    nc.vector.tensor_scalar(r1[:], m1[:], 1.0, None, AL.is_gt)
                    nc.vector.scalar_tensor_tensor(
                        m1[:], m1[:], be1, cur1T[:], AL.mult, AL.add)
                    nc.vector.tensor_sub(m1[:], m1[:], r1[:])
                    s1 = p5.tile([128, 128], F32R, tag="s1")
                    nc.vector.tensor_scalar(s1[:], m1[:], 1.0, None, AL.is_gt)
                    ps2 = psB.tile([128, 64], F32, tag="b")
                    nc.tensor.matmul(ps2[:], w2a_sb[:],
                                     s1[:, 0:64],
                                     start=True, stop=False)
                    nc.tensor.matmul(ps2[:], w2b_sb[:],
                                     s1[:, 64:128],
                                     start=False, stop=True)
                    r2 = p5.tile([128, 64], F32, tag="r2")
                    nc.vector.tensor_scalar(r2[:], m2[:], 1.0, None, AL.is_gt)
                    nc.vector.scalar_tensor_tensor(
                        m2[:], m2[:], be2, ps2[:], AL.mult, AL.add)
                    nc.vector.tensor_sub(m2[:], m2[:], r2[:])
                    nc.vector.tensor_scalar(m2[:], m2[:], b2_sb[:, 0:1],
                                            None, AL.add)
                    s2 = p5.tile([128, 64], F32R, tag="s2")
                    nc.vector.tensor_scalar(s2[:], m2[:], 1.0, None, AL.is_gt)
                    ps3 = psB.tile([35, 64], F32, tag="b")
                    nc.tensor.matmul(ps3[:], w3t_sb[:],
                                     s2[:],
                                     start=True, stop=True)
                    r3 = p5.tile([35, 64], F32, tag="r3")
                    nc.vector.tensor_scalar(r3[:], m3[:], 1.0, None, AL.is_gt)
                    nc.vector.scalar_tensor_tensor(
                        m3[:], m3[:], be3, ps3[:], AL.mult, AL.add)
                    nc.vector.tensor_sub(m3[:], m3[:], r3[:])
                    nc.vector.tensor_scalar(m3[:], m3[:], b3_sb[:, 0:1],
                                            None, AL.add)
                    nc.vector.tensor_scalar(outsb[:, t * 64:(t + 1) * 64],
                                            m3[:], 1.0, None, AL.is_gt)
                nc.sync.dma_start(
                    rap(out, 0, [[1, 35], [BC * 35, 25], [35, 64]]), outsb[:])
            if DBG:
                nc.sync.dma_start(dxm.ap(), xm.ap())
                nc.sync.dma_start(df1.ap(), f1p.ap())
                nc.sync.dma_start(dft.ap(), featd.ap())
                nc.sync.dma_start(dc1.ap(), cur1T[:])

    split_multi_waits(nc)
    return nc


def prep(inputs):
    f = np.float32
    w1 = np.asarray(inputs["conv1_w"], f)
    s1v = np.asarray(inputs["bn1_g"], f) / np.sqrt(
        np.asarray(inputs["bn1_v"], f) + 1e-5)
    sh1 = np.asarray(inputs["bn1_b"], f) + (
        np.asarray(inputs["conv1_b"], f) - np.asarray(inputs["bn1_m"], f)) * s1v
    w2 = np.asarray(inputs["conv2_w"], f)
    s2v = np.asarray(inputs["bn2_g"], f) / np.sqrt(
        np.asarray(inputs["bn2_v"], f) + 1e-5)
    sh2 = np.asarray(inputs["bn2_b"], f) + (
        np.asarray(inputs["conv2_b"], f) - np.asarray(inputs["bn2_m"], f)) * s2v

    w1b = np.zeros((72, 128), f)
    for bg in range(8):
        for ch in range(16):
            for dh in range(3):
                for dw in range(3):
                    w1b[dh * 24 + dw * 8 + bg, bg * 16 + ch] = \
                        w1[ch, 0, dh, dw] / 25.0
    bn1sv = np.tile(s1v, 8).astype(f)
    bn1bv = np.tile(sh1, 8).astype(f)

    w2t9 = np.zeros((9, 64, 128), f)
    for ti in range(9):
        dh, dw = ti // 3, ti % 3
        for bg in range(4):
            for ci in range(16):
                for co in range(32):
                    w2t9[ti, bg * 16 + ci, bg * 32 + co] = w2[co, ci, dh, dw]
    bn2sv = np.tile(s2v, 4).astype(f)
    bn2bv = np.tile(sh2, 4).astype(f)

    return dict(
        w1b=w1b, bn1s=bn1sv, bn1b=bn1bv,
        w2t=w2t9, bn2s=bn2sv, bn2b=bn2bv,
        w1t=np.ascontiguousarray(np.asarray(inputs["fc1_w"], f).T),
        b1=np.asarray(inputs["fc1_b"], f),
        w2a=np.ascontiguousarray(np.asarray(inputs["fc2_w"], f).T[0:128]),
        w2b=np.ascontiguousarray(np.asarray(inputs["fc2_w"], f).T[128:256]),
        b2=np.asarray(inputs["fc2_b"], f),
        w3t=np.ascontiguousarray(np.asarray(inputs["fc3_w"], f).T),
        b3=np.asarray(inputs["fc3_b"], f),
        ident=np.eye(64, dtype=f),
    )


def kernel(**inputs):
    f = np.float32
    x = np.asarray(inputs["x"], f)
    be1 = float(np.clip(np.asarray(inputs["beta1"], f), 0.0, 1.0))
    be2 = float(np.clip(np.asarray(inputs["beta2"], f), 0.0, 1.0))
    be3 = float(np.clip(np.asarray(inputs["beta3"], f), 0.0, 1.0))
    consts = prep(inputs)
    nc = build(be1, be2, be3)
    in_maps = []
    for c in range(NCORE):
        m = {"x": np.ascontiguousarray(x[:, c * BC:(c + 1) * BC])}
        m.update(consts)
        in_maps.append(m)
    res = bass_utils.run_bass_kernel_spmd(nc, in_maps, core_ids=list(range(NCORE)))
    return np.concatenate([res.results[c]["out"] for c in range(NCORE)], axis=1)



# revision 23
# speedup vs baseline: 5082.7109x; 1.4147x over previous
"""KWS-SNN Trainium2 kernel: 8-way batch-parallel Bass/Tile implementation.

Per core (BC=64): mean over T (DMA-fed vector adds) -> padded image to DRAM
-> conv1 (block-diag batch-packed, K=72, fp32r) with single-op 2x2 PSUM
pool-reduce -> BN+ReLU into SBUF-resident padded f1 -> conv2 (9 tap-matmuls,
K=64, fp32r) pool-reduce -> BN+ReLU -> featd round trip -> fc1 GEMM against
SBUF-preloaded weights (fp32r) -> transpose -> 25-step LIF scan -> spikes.
DMA is spread across the two HWDGE queues (sync/scalar) + gpsimd SWDGE.
"""
import sys
sys.path.insert(0, '/opt/trn_rl_repo')
import numpy as np
import concourse.bass as bass
import concourse.mybir as mybir
import bass_rust
from concourse.tile import TileContext
from concourse import bass_utils

F32 = mybir.dt.float32
F32R = mybir.dt.float32r
AL = mybir.AluOpType
AF = mybir.ActivationFunctionType
AX = mybir.AxisListType

T, BF, H, W = 25, 512, 100, 64
NCORE = 8
BC = BF // NCORE          # 64 batches per core
NCL = 35

XMP_B = 102 * 66          # 6732 per-batch padded mean image


def rap(handle, off, dims, dt=None):
    a = handle.ap()
    if dt is not None:
        a = a.bitcast(dt)
    return bass_rust.AP(tensor=a.tensor, offset=off, ap=[list(d) for d in dims])


def split_multi_waits(nc, max_waits=1):
    """This walrus build rejects >1 sync-wait per instruction; hoist excess
    waits onto same-engine NoOps inserted immediately before."""
    ctr = 0
    for f in nc.m.functions:
        for bb in f.blocks:
            il = bb.instructions
            if not any(i.sync_info is not None and len(i.sync_info.on_wait) > max_waits
                       for i in il):
                continue
            new = []
            for inst in il:
                si = inst.sync_info
                if si is not None and len(si.on_wait) > max_waits:
                    waits = list(si.on_wait)
                    while len(waits) > max_waits:
                        w = waits.pop(0)
                        nop = mybir.InstNoOp(
                            name=f"_ws_{ctr}", engine=inst.engine,
                            sync_info=mybir.SyncInfo(on_wait=[w], on_update=[]),
                            bass_nofuse=True)
                        ctr += 1
                        new.append(nop)
                    inst.sync_info = mybir.SyncInfo(
                        on_wait=waits, on_update=list(si.on_update))
                new.append(inst)
            bb.instructions = new
    return ctr


def build(be1, be2, be3):
    nc = bass.Bass()
    xin = nc.dram_tensor("x", [T, BC, H, W], F32, kind="ExternalInput")
    w1b = nc.dram_tensor("w1b", [72, 128], F32, kind="ExternalInput")
    bn1s = nc.dram_tensor("bn1s", [128], F32, kind="ExternalInput")
    bn1b = nc.dram_tensor("bn1b", [128], F32, kind="ExternalInput")
    w2t = nc.dram_tensor("w2t", [9, 64, 128], F32, kind="ExternalInput")
    bn2s = nc.dram_tensor("bn2s", [128], F32, kind="ExternalInput")
    bn2b = nc.dram_tensor("bn2b", [128], F32, kind="ExternalInput")
    w1t = nc.dram_tensor("w1t", [12800, 256], F32, kind="ExternalInput")
    b1 = nc.dram_tensor("b1", [256], F32, kind="ExternalInput")
    w2a = nc.dram_tensor("w2a", [128, 128], F32, kind="ExternalInput")
    w2b = nc.dram_tensor("w2b", [128, 128], F32, kind="ExternalInput")
    b2 = nc.dram_tensor("b2", [128], F32, kind="ExternalInput")
    w3t = nc.dram_tensor("w3t", [128, 35], F32, kind="ExternalInput")
    b3 = nc.dram_tensor("b3", [35], F32, kind="ExternalInput")
    ident = nc.dram_tensor("ident", [64, 64], F32, kind="ExternalInput")

    xmp = nc.dram_tensor("xmp", [BC * XMP_B], F32, kind="Internal")
    featd = nc.dram_tensor("featd", [BC * 12800], F32, kind="Internal")
    out = nc.dram_tensor("out", [T, BC, NCL], F32, kind="ExternalOutput")

    with TileContext(nc) as tc:
        with (
            tc.tile_pool(name="const", bufs=1) as pc,
            tc.tile_pool(name="main", bufs=1) as pm,
            tc.tile_pool(name="psA", bufs=4, space="PSUM") as psA,
            tc.tile_pool(name="psB", bufs=4, space="PSUM") as psB,
        ):
            # ---- constants to SBUF ----
            w1b_sb = pc.tile([72, 128], F32R, tag="w1b")
            nc.sync.dma_start(w1b_sb[:], w1b.ap().bitcast(F32R))
            # conv2 weights duplicated into both partition halves so either
            # f1pad 64-slice can be the matmul rhs (equal base-partition rule)
            w2t_sb = pc.tile([128, 9 * 128], F32R, tag="w2t")
            nc.sync.dma_start(
                w2t_sb[0:64, :], rap(w2t, 0, [[128, 64], [8192, 9], [1, 128]], F32R))
            nc.sync.dma_start(
                w2t_sb[64:128, :], rap(w2t, 0, [[128, 64], [8192, 9], [1, 128]], F32R))
            bn1s_sb = pc.tile([128, 1], F32, tag="b1s")
            bn1b_sb = pc.tile([128, 1], F32, tag="b1b")
            bn2s_sb = pc.tile([128, 1], F32, tag="b2s")
            bn2b_sb = pc.tile([128, 1], F32, tag="b2b")
            for sb, dr in ((bn1s_sb, bn1s), (bn1b_sb, bn1b),
                           (bn2s_sb, bn2s), (bn2b_sb, bn2b)):
                nc.scalar.dma_start(sb[:], rap(dr, 0, [[1, 128], [1, 1]]))
            w2a_sb = pc.tile([128, 128], F32R, tag="w2a")
            nc.scalar.dma_start(w2a_sb[:], w2a.ap().bitcast(F32R))
            w2b_sb = pc.tile([128, 128], F32R, tag="w2b")
            nc.scalar.dma_start(w2b_sb[:], w2b.ap().bitcast(F32R))
            w3t_sb = pc.tile([128, 35], F32R, tag="w3t")
            nc.scalar.dma_start(w3t_sb[:], w3t.ap().bitcast(F32R))
            b1_sb = pc.tile([128, 2], F32, tag="fb1")
            nc.scalar.dma_start(b1_sb[:], rap(b1, 0, [[1, 128], [128, 2]]))
            b2_sb = pc.tile([128, 1], F32, tag="fb2")
            nc.scalar.dma_start(b2_sb[:], rap(b2, 0, [[1, 128], [1, 1]]))
            b3_sb = pc.tile([35, 1], F32, tag="fb3")
            nc.scalar.dma_start(b3_sb[:], rap(b3, 0, [[1, 35], [1, 1]]))
            id_sb = pc.tile([64, 64], F32, tag="id")
            nc.scalar.dma_start(id_sb[:], ident.ap())

            cur1T = pm.tile([128, 128], F32, tag="cur1T")
            outsb = pm.tile([35, T * 64], F32, tag="outsb")

            # ---- phase A: sum over T (mean /25 folded into conv1 w) ----
            with (
                tc.tile_pool(name="phZ", bufs=1) as pz,
                tc.tile_pool(name="phA", bufs=4) as pa,
            ):
                # zero-fill padded image surface early
                zt = pz.tile([128, 3366], F32, tag="zt")
                nc.gpsimd.memset(zt[:], 0.0)
                nc.sync.dma_start(
                    rap(xmp, 0, [[3366, 128], [1, 3366]]), zt[:])
                acc = pz.tile([128, 3200], F32, tag="acc")
                for t in range(T):
                    xt = pa.tile([128, 3200], F32, tag="xt")
                    eng = nc.sync if t % 2 == 0 else nc.scalar
                    eng.dma_start(
                        xt[:], rap(xin, t * 409600, [[3200, 128], [1, 3200]]))
                    if t == 0:
                        nc.vector.tensor_copy(acc[:], xt[:])
                    else:
                        nc.vector.tensor_add(acc[:], acc[:], xt[:])
                # scatter sum into padded per-batch images (one DMA)
                nc.sync.dma_start(
                    rap(xmp, 67, [[XMP_B, 64], [3300, 2], [66, 50], [1, 64]]),
                    acc[:].rearrange("(b t) f -> b t f", t=2))

            # ---- conv1 + conv2, pipelined per 8-batch chunk ----
            with (
                tc.tile_pool(name="phC", bufs=2) as p1,
                tc.tile_pool(name="phD", bufs=2) as p2,
                tc.tile_pool(name="phF1", bufs=2) as pf,
            ):
                rblk = [(0, 14), (14, 12), (26, 12), (38, 12)]
                for c in range(8):
                    im1 = p1.tile([72, 6400], F32R, tag="im1")
                    for dh in range(3):
                        for dw in range(3):
                            eng = nc.sync if (dh * 3 + dw) % 2 == 0 else nc.scalar
                            eng.dma_start(
                                im1[dh * 24 + dw * 8:dh * 24 + dw * 8 + 8, :],
                                rap(xmp, c * 8 * XMP_B + dh * 66 + dw,
                                    [[XMP_B, 8], [66, 100], [1, 64]], F32R))
                    f1pad = pf.tile([128, 1768], F32R, tag="f1pad")
                    nc.gpsimd.memset(f1pad[:].bitcast(F32), 0.0)
                    hp = p1.tile([128, 1600], F32, tag="hp")
                    for s in range(13):
                        n = 512 if s < 12 else 256
                        rows = 8 if s < 12 else 4
                        ps = psA.tile([128, 512], F32, tag="cv")
                        nc.tensor.matmul(
                            ps[:, 0:n], w1b_sb[:],
                            im1[:, s * 512:s * 512 + n],
                            start=True, stop=True)
                        pv = ps[:, 0:n].rearrange(
                            "p (ro t1 wo t2) -> p ro wo t1 t2",
                            t1=2, wo=32, t2=2)
                        nc.vector.tensor_reduce(
                            hp[:, s * 128:s * 128 + rows * 16].rearrange(
                                "p (ro wo) -> p ro wo", wo=32),
                            pv, AX.XY, AL.max)
                    fv = f1pad[:].rearrange("p (r w) -> p r w", w=34)
                    nc.scalar.activation(
                        fv[:, 1:51, 1:33],
                        hp[:].rearrange("p (r w) -> p r w", w=32),
                        AF.Relu, bias=bn1b_sb[:, 0:1], scale=bn1s_sb[:, 0:1])

                    for g2 in range(2):
                        g = c * 2 + g2
                        fqv = f1pad[g2 * 64:(g2 + 1) * 64, :].rearrange(
                            "p (r w) -> p r w", w=34)
                        p2f = p2.tile([128, 400], F32, tag="p2f")
                        for (r0, nr) in rblk:
                            ps = psA.tile([128, 512], F32, tag="cv")
                            n = nr * 32
                            for ti in range(9):
                                dh, dw = ti // 3, ti % 3
                                nc.tensor.matmul(
                                    ps[:, 0:n],
                                    w2t_sb[g2 * 64:(g2 + 1) * 64,
                                           ti * 128:(ti + 1) * 128],
                                    fqv[:, dh + r0:dh + r0 + nr, dw:dw + 32],
                                    start=(ti == 0), stop=(ti == 8))
                            pv = ps[:, 0:n].rearrange(
                                "p (ro t1 wo t2) -> p ro wo t1 t2",
                                t1=2, wo=16, t2=2)
                            nc.vector.tensor_reduce(
                                p2f[:, (r0 // 2) * 16:((r0 + nr) // 2) * 16]
                                .rearrange("p (ro wo) -> p ro wo", wo=16),
                                pv, AX.XY, AL.max)
                        p2a = p2.tile([128, 400], F32, tag="p2a")
                        nc.scalar.activation(p2a[:], p2f[:], AF.Relu,
                                             bias=bn2b_sb[:, 0:1],
                                             scale=bn2s_sb[:, 0:1])
                        eng = nc.sync if g % 2 == 0 else nc.scalar
                        eng.dma_start(
                            rap(featd, g * 4 * 12800,
                                [[12800, 4], [400, 32], [1, 400]]),
                            p2a[:])

            # ---- fc1 GEMM: K=12800 in 13 super-chunks of 8 k-tiles ----
            with tc.tile_pool(name="phE", bufs=3) as p4:
                psf = psB.tile([64, 256], F32, tag="b")
                for kk in range(13):
                    nt = 8 if kk < 12 else 4
                    wt8 = p4.tile([128, 8 * 256], F32R, tag="wt8")
                    nc.scalar.dma_start(
                        wt8[:, 0:nt * 256],
                        rap(w1t, kk * 8 * 32768,
                            [[256, 128], [32768, nt], [1, 256]], F32R))
                    for j in range(nt):
                        k = kk * 8 + j
                        ft = p4.tile([128, 64], F32R, tag="ft", bufs=8)
                        nc.sync.dma_start(
                            ft[:], rap(featd, k * 128,
                                       [[1, 128], [12800, 64]], F32R))
                        nc.tensor.matmul(psf[:], ft[:],
                                         wt8[:, j * 256:(j + 1) * 256],
                                         start=(k == 0), stop=(k == 99))
                cur1 = p4.tile([64, 256], F32, tag="cur1")
                nc.scalar.copy(cur1[:], psf[:])
                for h in range(2):
                    pst = psB.tile([128, 64], F32, tag="b")
                    nc.tensor.transpose(pst[:], cur1[:, h * 128:(h + 1) * 128],
                                        id_sb[:])
                    nc.vector.tensor_scalar(cur1T[:, h * 64:(h + 1) * 64],
                                            pst[:], b1_sb[:, h:h + 1], None,
                                            AL.add)

            # ---- LIF scan (reset_t == spike_{t-1}; biases on scalar engine) ----
            with tc.tile_pool(name="phF", bufs=3) as p5:
                m1 = pm.tile([128, 128], F32, tag="m1")
                m2 = pm.tile([128, 64], F32, tag="m2")
                m3 = pm.tile([35, 64], F32, tag="m3")
                s1z = pm.tile([128, 128], F32R, tag="s1z")
                s2z = pm.tile([128, 64], F32R, tag="s2z")
                nc.gpsimd.memset(m1[:], 0.0)
                nc.gpsimd.memset(m2[:], 0.0)
                nc.gpsimd.memset(m3[:], 0.0)
                nc.gpsimd.memset(s1z[:].bitcast(F32), 0.0)
                nc.gpsimd.memset(s2z[:].bitcast(F32), 0.0)
                s1_prev, s2_prev = s1z, s2z
                for t in range(T):
                    # layer 1: m1 = be1*m1 + (cur1T - s1_prev); s1 = m1>1
                    t1 = p5.tile([128, 128], F32, tag="t1")
                    nc.vector.scalar_tensor_tensor(
                        t1[:], s1_prev[:].bitcast(F32), -1.0, cur1T[:],
                        AL.mult, AL.add)
                    nc.vector.scalar_tensor_tensor(
                        m1[:], m1[:], be1, t1[:], AL.mult, AL.add)
                    s1 = p5.tile([128, 128], F32R, tag="s1")
                    nc.vector.tensor_scalar(s1[:], m1[:], 1.0, None, AL.is_gt)
                    ps2 = psB.tile([128, 64], F32, tag="b")
                    nc.tensor.matmul(ps2[:], w2a_sb[:],
                                     s1[:, 0:64],
                                     start=True, stop=False)
                    nc.tensor.matmul(ps2[:], w2b_sb[:],
                                     s1[:, 64:128],
                                     start=False, stop=True)
                    # layer 2: in2 = ps2 + b2 - s2_prev (scalar engine adds bias)
                    t2 = p5.tile([128, 64], F32, tag="t2")
                    nc.scalar.activation(t2[:], ps2[:], AF.Identity,
                                         bias=b2_sb[:, 0:1], scale=1.0)
                    nc.vector.scalar_tensor_tensor(
                        t2[:], s2_prev[:].bitcast(F32), -1.0, t2[:],
                        AL.mult, AL.add)
                    nc.vector.scalar_tensor_tensor(
                        m2[:], m2[:], be2, t2[:], AL.mult, AL.add)
                    s2 = p5.tile([128, 64], F32R, tag="s2")
                    nc.vector.tensor_scalar(s2[:], m2[:], 1.0, None, AL.is_gt)
                    ps3 = psB.tile([35, 64], F32, tag="b")
                    nc.tensor.matmul(ps3[:], w3t_sb[:],
                                     s2[:],
                                     start=True, stop=True)
                    # layer 3: m3 = be3*m3 + (ps3 + b3 - s3_prev)
                    t3 = p5.tile([35, 64], F32, tag="t3")
                    nc.scalar.activation(t3[:], ps3[:], AF.Identity,
                                         bias=b3_sb[:, 0:1], scale=1.0)
                    s3prev = outsb[:, (t - 1) * 64:t * 64] if t > 0 else outsb[:, 0:64]
                    if t > 0:
                        nc.vector.tensor_sub(t3[:], t3[:], s3prev)
                    nc.vector.scalar_tensor_tensor(
                        m3[:], m3[:], be3, t3[:], AL.mult, AL.add)
                    nc.vector.tensor_scalar(outsb[:, t * 64:(t + 1) * 64],
                                            m3[:], 1.0, None, AL.is_gt)
                    s1_prev, s2_prev = s1, s2
                nc.sync.dma_start(
                    rap(out, 0, [[1, 35], [BC * 35, 25], [35, 64]]), outsb[:])

    split_multi_waits(nc)
    return nc


def prep(inputs):
    f = np.float32
    w1 = np.asarray(inputs["conv1_w"], f)
    s1v = np.asarray(inputs["bn1_g"], f) / np.sqrt(
        np.asarray(inputs["bn1_v"], f) + 1e-5)
    sh1 = np.asarray(inputs["bn1_b"], f) + (
        np.asarray(inputs["conv1_b"], f) - np.asarray(inputs["bn1_m"], f)) * s1v
    w2 = np.asarray(inputs["conv2_w"], f)
    s2v = np.asarray(inputs["bn2_g"], f) / np.sqrt(
        np.asarray(inputs["bn2_v"], f) + 1e-5)
    sh2 = np.asarray(inputs["bn2_b"], f) + (
        np.asarray(inputs["conv2_b"], f) - np.asarray(inputs["bn2_m"], f)) * s2v

    w1b = np.zeros((72, 128), f)
    for bg in range(8):
        for ch in range(16):
            for dh in range(3):
                for dw in range(3):
                    w1b[dh * 24 + dw * 8 + bg, bg * 16 + ch] = \
                        w1[ch, 0, dh, dw] / 25.0
    bn1sv = np.tile(s1v, 8).astype(f)
    bn1bv = np.tile(sh1, 8).astype(f)

    w2t9 = np.zeros((9, 64, 128), f)
    for ti in range(9):
        dh, dw = ti // 3, ti % 3
        for bg in range(4):
            for ci in range(16):
                for co in range(32):
                    w2t9[ti, bg * 16 + ci, bg * 32 + co] = w2[co, ci, dh, dw]
    bn2sv = np.tile(s2v, 4).astype(f)
    bn2bv = np.tile(sh2, 4).astype(f)

    return dict(
        w1b=w1b, bn1s=bn1sv, bn1b=bn1bv,
        w2t=w2t9, bn2s=bn2sv, bn2b=bn2bv,
        w1t=np.ascontiguousarray(np.asarray(inputs["fc1_w"], f).T),
        b1=np.asarray(inputs["fc1_b"], f),
        w2a=np.ascontiguousarray(np.asarray(inputs["fc2_w"], f).T[0:128]),
        w2b=np.ascontiguousarray(np.asarray(inputs["fc2_w"], f).T[128:256]),
        b2=np.asarray(inputs["fc2_b"], f),
        w3t=np.ascontiguousarray(np.asarray(inputs["fc3_w"], f).T),
        b3=np.asarray(inputs["fc3_b"], f),
        ident=np.eye(64, dtype=f),
    )


def kernel(**inputs):
    f = np.float32
    x = np.asarray(inputs["x"], f)
    be1 = float(np.clip(np.asarray(inputs["beta1"], f), 0.0, 1.0))
    be2 = float(np.clip(np.asarray(inputs["beta2"], f), 0.0, 1.0))
    be3 = float(np.clip(np.asarray(inputs["beta3"], f), 0.0, 1.0))
    consts = prep(inputs)
    nc = build(be1, be2, be3)
    in_maps = []
    for c in range(NCORE):
        m = {"x": np.ascontiguousarray(x[:, c * BC:(c + 1) * BC])}
        m.update(consts)
        in_maps.append(m)
    res = bass_utils.run_bass_kernel_spmd(nc, in_maps, core_ids=list(range(NCORE)))
    return np.concatenate([res.results[c]["out"] for c in range(NCORE)], axis=1)
